# revision 36
# baseline (speedup 1.0000x reference)
"""Trainium2 Bass kernel for nn_ConditionalLayer (MoE-style conditional FC).

Reference semantics (N=16384 rows, D=512 features, C=8 conditions):
    out[n] = sum_c relu( (x[n] * [cond_ids[n]==c]) @ W_c + b_c )
           = relu(x[n] @ W_{c*} + b_{c*}) + sum_{c != c*} relu(b_c)
where c* = cond_ids[n].  Masked-out rows still contribute relu(b_c), a
per-condition constant vector corr_{c*} = S - relu(b_{c*}), S = sum_c relu(b_c).

Strategy (expert-parallel, 8 cores == 8 conditions):
  - Host: group rows by condition (argsort), pad each group to a common CAP,
    and ship core c the transposed row-block xT_c = [D, CAP] plus W_c and the
    two per-feature constant vectors (b_c, corr_c).
  - Device (per core): yT = relu(W_c.T-contract xT + b_c) + corr_c as a dense
    [CAP,512]x[512,512] matmul tiled on the 128x128 PE, PSUM fp32 accumulation,
    relu+bias on ScalarE, +corr on VectorE, all overlapped with DMA.
  - Host: scatter rows back to their original positions.

Each row is matmul'd against only its own condition's weights: 1/8th the FLOPs
of the reference's masked-batch formulation.
"""

import math

import numpy as np

N, D, C = 16384, 512, 8
NCORES = 8
P = 128
KT = D // P  # 4 k-tiles
FT = D // P  # 4 output feature tiles
RTILE = 512  # moving free dim per matmul (max for fp32)

# MODE:
#   "f32"         — plain fp32 matmul (4 cycles/row, safest numerics)
#   "f32r_direct" — fp32r matmul, operands DMA'd as raw fp32 bits (1 cycle/row)
#   "f32r_round"  — fp32r matmul, operands pre-rounded via DVE copy
MODE = "f32r_direct"

_PROGRAM_CACHE: dict = {}


def _build_program(cap: int, mode: str):
    import concourse.mybir as mybir
    import concourse.tile as tile
    from concourse import bacc

    f32 = mybir.dt.float32
    mmdt = f32 if mode == "f32" else mybir.dt.float32r

    # Bacc (not Bass): its compile() runs move_matmul_waits_to_ldweights +
    # generate_event_semaphores, required to satisfy the 1-wait-per-instruction
    # HW constraint on fused-weight-load matmuls (fp32/fp32r).
    nc = bacc.Bacc("TRN2", target_bir_lowering=False, debug=False)

    xt = nc.dram_tensor("xt", [D, cap], f32, kind="ExternalInput")
    w = nc.dram_tensor("w", [D, D], f32, kind="ExternalInput")
    b2 = nc.dram_tensor("b2", [P, FT], f32, kind="ExternalInput")
    corr2 = nc.dram_tensor("corr2", [P, FT], f32, kind="ExternalInput")
    yt = nc.dram_tensor("yt", [D, cap], f32, kind="ExternalOutput")

    xt_r = xt[:].rearrange("(kt p) r -> p kt r", p=P)  # [128, KT, cap]
    w_r = w[:].rearrange("(kt p) f -> p kt f", p=P)  # [128, KT, D]
    yt_r = yt[:].rearrange("(ft p) r -> p ft r", p=P)  # [128, FT, cap]

    # r-tile schedule: small tiles at BOTH ends — the first so the PE starts
    # as soon as a small x slice lands, the last so the final
    # compute->store chain drains quickly — with full 512s in the middle.
    # cap is a multiple of 128.
    head, tail = [256, 384], []
    if cap >= sum(head) + sum(tail) + RTILE:
        mid = cap - sum(head) - sum(tail)
        sizes = head + [RTILE] * (mid // RTILE)
        if mid % RTILE:
            sizes.append(mid % RTILE)
        sizes += tail
    else:
        sizes = []
        rem = cap
        while rem:
            s = min(RTILE, rem)
            sizes.append(s)
            rem -= s
    r_tiles = []
    off = 0
    for s in sizes:
        r_tiles.append((off, s))
        off += s
    assert off == cap

    direct = mode != "f32r_round"
    load_dt = mmdt if direct else f32

    with tile.TileContext(nc) as tc:
        with (
            tc.tile_pool(name="wpool", bufs=1) as wpool,
            tc.tile_pool(name="cpool", bufs=1) as cpool,
            tc.tile_pool(name="xpool", bufs=4) as xpool,
            tc.tile_pool(name="opool", bufs=3) as opool,
            tc.tile_pool(name="pspool", bufs=8, space="PSUM") as pspool,
        ):
            # Per-queue transfers are FIFO; cross-queue they round-robin.
            # The first matmul group (r-tile 0, ft 0) only needs the ft=0
            # quarter of w plus the first x slice: put w_ft0 first on the
            # Sync queue (ahead of the x-load FIFO) and the other three
            # quarters on GpSimd's queue so they stream in parallel without
            # delaying the x prefetches.  Consts + stores also go on GpSimd
            # so load-prefetch and store-drain don't serialize on one
            # trigger queue (each dma trigger costs ~600-900ns of
            # issuing-engine time).
            w_sb = wpool.tile([P, KT, D], load_dt)
            src = w_r if load_dt is f32 else w_r.bitcast(load_dt)
            nc.sync.dma_start(w_sb[:, :, 0:P], src[:, :, 0:P])
            nc.gpsimd.dma_start(w_sb[:, :, P:D], src[:, :, P:D])
            if not direct:
                w_rnd = wpool.tile([P, KT, D], mmdt)
                nc.vector.tensor_copy(w_rnd[:], w_sb[:])
                w_mm = w_rnd
            else:
                w_mm = w_sb
            b_sb = cpool.tile([P, FT], f32)
            nc.gpsimd.dma_start(b_sb[:], b2[:])
            c_sb = cpool.tile([P, FT], f32)
            nc.gpsimd.dma_start(c_sb[:], corr2[:])

            store_engines = [nc.gpsimd, nc.scalar]
            for ri, (roff, rsz) in enumerate(r_tiles):
                x_sb = xpool.tile([P, KT, RTILE], load_dt, tag="x")
                xsrc = xt_r[:, :, roff : roff + rsz]
                if load_dt is not f32:
                    xsrc = xsrc.bitcast(load_dt)
                nc.sync.dma_start(x_sb[:, :, :rsz], xsrc)
                if not direct:
                    x_rnd = xpool.tile([P, KT, RTILE], mmdt, tag="xr")
                    nc.vector.tensor_copy(x_rnd[:, :, :rsz], x_sb[:, :, :rsz])
                    x_mm = x_rnd
                else:
                    x_mm = x_sb
                o_sb = opool.tile([P, FT, RTILE], f32, tag="o")
                for ft in range(FT):
                    ps = pspool.tile([P, RTILE], f32, tag="ps")
                    for kt in range(KT):
                        nc.tensor.matmul(
                            ps[:, :rsz],
                            lhsT=w_mm[:, kt, ft * P : (ft + 1) * P],
                            rhs=x_mm[:, kt, :rsz],
                            start=(kt == 0),
                            stop=(kt == KT - 1),
                        )
                    nc.scalar.activation(
                        o_sb[:, ft, :rsz],
                        ps[:, :rsz],
                        mybir.ActivationFunctionType.Relu,
                        bias=b_sb[:, ft : ft + 1],
                    )
                    nc.vector.tensor_scalar_add(
                        o_sb[:, ft, :rsz], o_sb[:, ft, :rsz], c_sb[:, ft : ft + 1]
                    )
                # one batched store per r-tile, alternating between the
                # GpSimd and Vector trigger queues — transfers are serviced
                # round-robin per queue, so a third queue raises aggregate
                # DMA throughput
                store_engines[ri % 2].dma_start(
                    yt_r[:, :, roff : roff + rsz], o_sb[:, :, :rsz]
                )

    nc.compile()
    return nc


def _get_program(cap: int, mm_dtype: str):
    key = (cap, mm_dtype)
    if key not in _PROGRAM_CACHE:
        _PROGRAM_CACHE[key] = _build_program(cap, mm_dtype)
    return _PROGRAM_CACHE[key]


def _route(x, cond_ids, W, b):
    """Host-side routing: group rows by condition, build per-core inputs."""
    x = np.ascontiguousarray(np.asarray(x, dtype=np.float32))
    cond_ids = np.asarray(cond_ids, dtype=np.int32)
    W = np.asarray(W, dtype=np.float32)
    b = np.asarray(b, dtype=np.float32)

    counts = np.bincount(cond_ids, minlength=C)
    cap = max(P, math.ceil(counts.max() / P) * P)
    order = np.argsort(cond_ids, kind="stable")
    starts = np.concatenate([[0], np.cumsum(counts)])

    relu_b = np.maximum(b, 0.0)  # [C, D]
    S = relu_b.sum(axis=0)  # [D]

    in_maps = []
    rows_per_core = []
    for c in range(C):
        rows_c = order[starts[c] : starts[c + 1]]
        rows_per_core.append(rows_c)
        xT_c = np.zeros((D, cap), dtype=np.float32)
        if len(rows_c):
            xT_c[:, : len(rows_c)] = x[rows_c].T
        corr_c = S - relu_b[c]
        in_maps.append(
            {
                "xt": xT_c,
                "w": np.ascontiguousarray(W[c]),
                "b2": np.ascontiguousarray(b[c].reshape(FT, P).T),
                "corr2": np.ascontiguousarray(corr_c.reshape(FT, P).T),
            }
        )
    return in_maps, rows_per_core, cap


def run(x, cond_ids, W, b, trace: bool = False):
    """Run the kernel; returns (out, BassKernelResults)."""
    try:
        from concourse.bass_utils import run_bass_kernel_spmd
    except ImportError:
        import sys

        sys.path.append("/opt/trn_rl_repo")
        from concourse.bass_utils import run_bass_kernel_spmd

    in_maps, rows_per_core, cap = _route(x, cond_ids, W, b)
    nc = _get_program(cap, MODE)
    res = run_bass_kernel_spmd(
        nc, in_maps, core_ids=list(range(NCORES)), trace=trace
    )

    out = np.empty((N, D), dtype=np.float32)
    for c in range(C):
        rows_c = rows_per_core[c]
        if len(rows_c):
            out[rows_c] = res.results[c]["yt"][:, : len(rows_c)].T
    return out, res


def kernel(x, cond_ids, W, b):
    out, _ = run(x, cond_ids, W, b, trace=False)
    return out


# revision 42
# speedup vs baseline: 1.0559x; 1.0559x over previous
"""Trainium2 Bass kernel for nn_ConditionalLayer (MoE-style conditional FC).

Reference semantics (N=16384 rows, D=512 features, C=8 conditions):
    out[n] = sum_c relu( (x[n] * [cond_ids[n]==c]) @ W_c + b_c )
           = relu(x[n] @ W_{c*} + b_{c*}) + sum_{c != c*} relu(b_c)
where c* = cond_ids[n].  Masked-out rows still contribute relu(b_c), a
per-condition constant vector corr_{c*} = S - relu(b_{c*}), S = sum_c relu(b_c).

Strategy (expert-parallel, 8 cores == 8 conditions):
  - Host: group rows by condition (argsort), pad each group to a common CAP,
    and ship core c the transposed row-block xT_c = [D, CAP] plus W_c and the
    two per-feature constant vectors (b_c, corr_c).
  - Device (per core): yT = relu(W_c.T-contract xT + b_c) + corr_c as a dense
    [CAP,512]x[512,512] matmul tiled on the 128x128 PE, PSUM fp32 accumulation,
    relu+bias on ScalarE, +corr on VectorE, all overlapped with DMA.
  - Host: scatter rows back to their original positions.

Each row is matmul'd against only its own condition's weights: 1/8th the FLOPs
of the reference's masked-batch formulation.
"""

import math

import numpy as np

N, D, C = 16384, 512, 8
NCORES = 8
P = 128
KT = D // P  # 4 k-tiles
FT = D // P  # 4 output feature tiles
RTILE = 512  # moving free dim per matmul (max for fp32)

# MODE:
#   "f32"         — plain fp32 matmul (4 cycles/row, safest numerics)
#   "f32r_direct" — fp32r matmul, operands DMA'd as raw fp32 bits (1 cycle/row)
#   "f32r_round"  — fp32r matmul, operands pre-rounded via DVE copy
MODE = "f32r_direct"

_PROGRAM_CACHE: dict = {}


def _build_program(cap: int, mode: str):
    import concourse.mybir as mybir
    import concourse.tile as tile
    from concourse import bacc

    f32 = mybir.dt.float32
    mmdt = f32 if mode == "f32" else mybir.dt.float32r

    # Bacc (not Bass): its compile() runs move_matmul_waits_to_ldweights +
    # generate_event_semaphores, required to satisfy the 1-wait-per-instruction
    # HW constraint on fused-weight-load matmuls (fp32/fp32r).
    nc = bacc.Bacc("TRN2", target_bir_lowering=False, debug=False)

    xt = nc.dram_tensor("xt", [D, cap], f32, kind="ExternalInput")
    w = nc.dram_tensor("w", [D, D], f32, kind="ExternalInput")
    b2 = nc.dram_tensor("b2", [P, FT], f32, kind="ExternalInput")
    corr2 = nc.dram_tensor("corr2", [P, FT], f32, kind="ExternalInput")
    yt = nc.dram_tensor("yt", [D, cap], f32, kind="ExternalOutput")

    xt_r = xt[:].rearrange("(kt p) r -> p kt r", p=P)  # [128, KT, cap]
    w_r = w[:].rearrange("(kt p) f -> p kt f", p=P)  # [128, KT, D]
    yt_r = yt[:].rearrange("(ft p) r -> p ft r", p=P)  # [128, FT, cap]

    # r-tile schedule: small tiles first (256, 384) so the PE starts as soon
    # as a small x slice lands, then full 512s.  (Small tiles at the tail
    # were measured net-negative: extra LDWEIGHTS + fp32r's 4-cycles/row
    # penalty below free-dim 256 outweigh the faster drain.)
    # cap is a multiple of 128.
    head, tail = [256, 384], []
    if cap >= sum(head) + sum(tail) + RTILE:
        mid = cap - sum(head) - sum(tail)
        sizes = head + [RTILE] * (mid // RTILE)
        if mid % RTILE:
            sizes.append(mid % RTILE)
        sizes += tail
    else:
        sizes = []
        rem = cap
        while rem:
            s = min(RTILE, rem)
            sizes.append(s)
            rem -= s
    r_tiles = []
    off = 0
    for s in sizes:
        r_tiles.append((off, s))
        off += s
    assert off == cap

    direct = mode != "f32r_round"
    load_dt = mmdt if direct else f32

    with tile.TileContext(nc) as tc:
        with (
            tc.tile_pool(name="wpool", bufs=1) as wpool,
            tc.tile_pool(name="cpool", bufs=1) as cpool,
            tc.tile_pool(name="xpool", bufs=4) as xpool,
            tc.tile_pool(name="opool", bufs=3) as opool,
            tc.tile_pool(name="pspool", bufs=8, space="PSUM") as pspool,
        ):
            # Per-queue transfers are FIFO; cross-queue they round-robin.
            # w goes FIRST on the Sync queue ahead of the x-load FIFO
            # (every matmul group needs all of it before its k-accumulation
            # completes); consts + stores go on GpSimd's queue so
            # load-prefetch and store-drain don't serialize on one trigger
            # queue (each dma trigger costs ~600-900ns of issuing-engine
            # time).
            w_sb = wpool.tile([P, KT, D], load_dt)
            src = w_r if load_dt is f32 else w_r.bitcast(load_dt)
            nc.sync.dma_start(w_sb[:], src)
            if not direct:
                w_rnd = wpool.tile([P, KT, D], mmdt)
                nc.vector.tensor_copy(w_rnd[:], w_sb[:])
                w_mm = w_rnd
            else:
                w_mm = w_sb
            b_sb = cpool.tile([P, FT], f32)
            nc.gpsimd.dma_start(b_sb[:], b2[:])
            c_sb = cpool.tile([P, FT], f32)
            nc.gpsimd.dma_start(c_sb[:], corr2[:])

            for roff, rsz in r_tiles:
                x_sb = xpool.tile([P, KT, RTILE], load_dt, tag="x")
                xsrc = xt_r[:, :, roff : roff + rsz]
                if load_dt is not f32:
                    xsrc = xsrc.bitcast(load_dt)
                nc.sync.dma_start(x_sb[:, :, :rsz], xsrc)
                if not direct:
                    x_rnd = xpool.tile([P, KT, RTILE], mmdt, tag="xr")
                    nc.vector.tensor_copy(x_rnd[:, :, :rsz], x_sb[:, :, :rsz])
                    x_mm = x_rnd
                else:
                    x_mm = x_sb
                o_sb = opool.tile([P, FT, RTILE], f32, tag="o")
                for ft in range(FT):
                    ps = pspool.tile([P, RTILE], f32, tag="ps")
                    for kt in range(KT):
                        nc.tensor.matmul(
                            ps[:, :rsz],
                            lhsT=w_mm[:, kt, ft * P : (ft + 1) * P],
                            rhs=x_mm[:, kt, :rsz],
                            start=(kt == 0),
                            stop=(kt == KT - 1),
                        )
                    nc.scalar.activation(
                        o_sb[:, ft, :rsz],
                        ps[:, :rsz],
                        mybir.ActivationFunctionType.Relu,
                        bias=b_sb[:, ft : ft + 1],
                    )
                    nc.vector.tensor_scalar_add(
                        o_sb[:, ft, :rsz], o_sb[:, ft, :rsz], c_sb[:, ft : ft + 1]
                    )
                # one batched store per r-tile on GpSimd's queue
                nc.gpsimd.dma_start(
                    yt_r[:, :, roff : roff + rsz], o_sb[:, :, :rsz]
                )

    nc.compile()
    return nc


def _get_program(cap: int, mode: str):
    key = (cap, mode)
    if key not in _PROGRAM_CACHE:
        _PROGRAM_CACHE[key] = _build_program(cap, mode)
    return _PROGRAM_CACHE[key]


def _route(x, cond_ids, W, b):
    """Host-side routing: group rows by condition, build per-core inputs."""
    x = np.ascontiguousarray(np.asarray(x, dtype=np.float32))
    cond_ids = np.asarray(cond_ids, dtype=np.int32)
    W = np.asarray(W, dtype=np.float32)
    b = np.asarray(b, dtype=np.float32)

    counts = np.bincount(cond_ids, minlength=C)
    cap = max(P, math.ceil(counts.max() / P) * P)
    order = np.argsort(cond_ids, kind="stable")
    starts = np.concatenate([[0], np.cumsum(counts)])

    relu_b = np.maximum(b, 0.0)  # [C, D]
    S = relu_b.sum(axis=0)  # [D]

    in_maps = []
    rows_per_core = []
    for c in range(C):
        rows_c = order[starts[c] : starts[c + 1]]
        rows_per_core.append(rows_c)
        xT_c = np.zeros((D, cap), dtype=np.float32)
        if len(rows_c):
            xT_c[:, : len(rows_c)] = x[rows_c].T
        corr_c = S - relu_b[c]
        in_maps.append(
            {
                "xt": xT_c,
                "w": np.ascontiguousarray(W[c]),
                "b2": np.ascontiguousarray(b[c].reshape(FT, P).T),
                "corr2": np.ascontiguousarray(corr_c.reshape(FT, P).T),
            }
        )
    return in_maps, rows_per_core, cap


def run(x, cond_ids, W, b, trace: bool = False):
    """Run the kernel; returns (out, BassKernelResults)."""
    try:
        from concourse.bass_utils import run_bass_kernel_spmd
    except ImportError:
        import sys

        sys.path.append("/opt/trn_rl_repo")
        from concourse.bass_utils import run_bass_kernel_spmd

    in_maps, rows_per_core, cap = _route(x, cond_ids, W, b)
    nc = _get_program(cap, MODE)
    res = run_bass_kernel_spmd(
        nc, in_maps, core_ids=list(range(NCORES)), trace=trace
    )

    out = np.empty((len(np.asarray(cond_ids)), D), dtype=np.float32)
    for c in range(C):
        rows_c = rows_per_core[c]
        if len(rows_c):
            out[rows_c] = res.results[c]["yt"][:, : len(rows_c)].T
    return out, res


def kernel(x, cond_ids, W, b):
    out, _ = run(x, cond_ids, W, b, trace=False)
    return out



# revision 44
# speedup vs baseline: 1.0732x; 1.0163x over previous
"""Trainium2 Bass kernel for nn_ConditionalLayer (MoE-style conditional FC).

Reference semantics (N=16384 rows, D=512 features, C=8 conditions):
    out[n] = sum_c relu( (x[n] * [cond_ids[n]==c]) @ W_c + b_c )
           = relu(x[n] @ W_{c*} + b_{c*}) + sum_{c != c*} relu(b_c)
where c* = cond_ids[n].  Masked-out rows still contribute relu(b_c), a
per-condition constant vector corr_{c*} = S - relu(b_{c*}), S = sum_c relu(b_c).

Strategy (expert-parallel, 8 cores == 8 conditions):
  - Host: group rows by condition (argsort), pad each group to a common CAP,
    and ship core c the transposed row-block xT_c = [D, CAP] plus W_c and the
    two per-feature constant vectors (b_c, corr_c).
  - Device (per core): yT = relu(W_c.T-contract xT + b_c) + corr_c as a dense
    [CAP,512]x[512,512] matmul tiled on the 128x128 PE, PSUM fp32 accumulation,
    relu+bias on ScalarE, +corr on VectorE, all overlapped with DMA.
  - Host: scatter rows back to their original positions.

Each row is matmul'd against only its own condition's weights: 1/8th the FLOPs
of the reference's masked-batch formulation.
"""

import math

import numpy as np

N, D, C = 16384, 512, 8
NCORES = 8
P = 128
KT = D // P  # 4 k-tiles
FT = D // P  # 4 output feature tiles
RTILE = 512  # moving free dim per matmul (max for fp32)

# MODE:
#   "f32"         — plain fp32 matmul (4 cycles/row, safest numerics)
#   "f32r_direct" — fp32r matmul, operands DMA'd as raw fp32 bits (1 cycle/row)
#   "f32r_round"  — fp32r matmul, operands pre-rounded via DVE copy
MODE = "f32r_direct"

_PROGRAM_CACHE: dict = {}


def _build_program(cap: int, mode: str):
    import concourse.mybir as mybir
    import concourse.tile as tile
    from concourse import bacc

    f32 = mybir.dt.float32
    mmdt = f32 if mode == "f32" else mybir.dt.float32r

    # Bacc (not Bass): its compile() runs move_matmul_waits_to_ldweights +
    # generate_event_semaphores, required to satisfy the 1-wait-per-instruction
    # HW constraint on fused-weight-load matmuls (fp32/fp32r).
    nc = bacc.Bacc("TRN2", target_bir_lowering=False, debug=False)

    xt = nc.dram_tensor("xt", [D, cap], f32, kind="ExternalInput")
    w = nc.dram_tensor("w", [D, D], f32, kind="ExternalInput")
    b2 = nc.dram_tensor("b2", [P, FT], f32, kind="ExternalInput")
    corr2 = nc.dram_tensor("corr2", [P, FT], f32, kind="ExternalInput")
    yt = nc.dram_tensor("yt", [D, cap], f32, kind="ExternalOutput")

    xt_r = xt[:].rearrange("(kt p) r -> p kt r", p=P)  # [128, KT, cap]
    w_r = w[:].rearrange("(kt p) f -> p kt f", p=P)  # [128, KT, D]
    yt_r = yt[:].rearrange("(ft p) r -> p ft r", p=P)  # [128, FT, cap]

    # r-tile schedule: small tiles first (256, 384) so the PE starts as soon
    # as a small x slice lands, then full 512s.  (Small tiles at the tail
    # were measured net-negative: extra LDWEIGHTS + fp32r's 4-cycles/row
    # penalty below free-dim 256 outweigh the faster drain.)
    # cap is a multiple of 128.
    head, tail = [256, 384], []
    if cap >= sum(head) + sum(tail) + RTILE:
        mid = cap - sum(head) - sum(tail)
        sizes = head + [RTILE] * (mid // RTILE)
        if mid % RTILE:
            sizes.append(mid % RTILE)
        sizes += tail
    else:
        sizes = []
        rem = cap
        while rem:
            s = min(RTILE, rem)
            sizes.append(s)
            rem -= s
    r_tiles = []
    off = 0
    for s in sizes:
        r_tiles.append((off, s))
        off += s
    assert off == cap

    direct = mode != "f32r_round"
    load_dt = mmdt if direct else f32

    with tile.TileContext(nc) as tc:
        with (
            tc.tile_pool(name="wpool", bufs=1) as wpool,
            tc.tile_pool(name="cpool", bufs=1) as cpool,
            tc.tile_pool(name="xpool", bufs=4) as xpool,
            tc.tile_pool(name="opool", bufs=3) as opool,
            tc.tile_pool(name="pspool", bufs=8, space="PSUM") as pspool,
        ):
            # Per-queue transfers are FIFO; cross-queue they round-robin.
            # w goes FIRST on the Sync queue ahead of the x-load FIFO
            # (every matmul group needs all of it before its k-accumulation
            # completes); consts + stores go on GpSimd's queue so
            # load-prefetch and store-drain don't serialize on one trigger
            # queue (each dma trigger costs ~600-900ns of issuing-engine
            # time).
            w_sb = wpool.tile([P, KT, D], load_dt)
            src = w_r if load_dt is f32 else w_r.bitcast(load_dt)
            nc.sync.dma_start(w_sb[:], src)
            if not direct:
                w_rnd = wpool.tile([P, KT, D], mmdt)
                nc.vector.tensor_copy(w_rnd[:], w_sb[:])
                w_mm = w_rnd
            else:
                w_mm = w_sb
            b_sb = cpool.tile([P, FT], f32)
            nc.gpsimd.dma_start(b_sb[:], b2[:])
            c_sb = cpool.tile([P, FT], f32)
            nc.gpsimd.dma_start(c_sb[:], corr2[:])

            for roff, rsz in r_tiles:
                x_sb = xpool.tile([P, KT, RTILE], load_dt, tag="x")
                xsrc = xt_r[:, :, roff : roff + rsz]
                if load_dt is not f32:
                    xsrc = xsrc.bitcast(load_dt)
                nc.sync.dma_start(x_sb[:, :, :rsz], xsrc)
                if not direct:
                    x_rnd = xpool.tile([P, KT, RTILE], mmdt, tag="xr")
                    nc.vector.tensor_copy(x_rnd[:, :, :rsz], x_sb[:, :, :rsz])
                    x_mm = x_rnd
                else:
                    x_mm = x_sb
                o_sb = opool.tile([P, FT, RTILE], f32, tag="o")
                for ft in range(FT):
                    ps = pspool.tile([P, RTILE], f32, tag="ps")
                    for kt in range(KT):
                        nc.tensor.matmul(
                            ps[:, :rsz],
                            lhsT=w_mm[:, kt, ft * P : (ft + 1) * P],
                            rhs=x_mm[:, kt, :rsz],
                            start=(kt == 0),
                            stop=(kt == KT - 1),
                        )
                    # relu(z + b) + corr == max(z + (b + corr), corr) since
                    # corr >= 0 — one DVE op instead of an ACT relu plus a
                    # DVE add (b_sb holds b + corr, c_sb holds corr)
                    nc.vector.tensor_scalar(
                        o_sb[:, ft, :rsz],
                        ps[:, :rsz],
                        b_sb[:, ft : ft + 1],
                        c_sb[:, ft : ft + 1],
                        mybir.AluOpType.add,
                        mybir.AluOpType.max,
                    )
                # one batched store per r-tile on GpSimd's queue
                nc.gpsimd.dma_start(
                    yt_r[:, :, roff : roff + rsz], o_sb[:, :, :rsz]
                )

    nc.compile()
    return nc


def _get_program(cap: int, mode: str):
    key = (cap, mode)
    if key not in _PROGRAM_CACHE:
        _PROGRAM_CACHE[key] = _build_program(cap, mode)
    return _PROGRAM_CACHE[key]


def _route(x, cond_ids, W, b):
    """Host-side routing: group rows by condition, build per-core inputs."""
    x = np.ascontiguousarray(np.asarray(x, dtype=np.float32))
    cond_ids = np.asarray(cond_ids, dtype=np.int32)
    W = np.asarray(W, dtype=np.float32)
    b = np.asarray(b, dtype=np.float32)

    counts = np.bincount(cond_ids, minlength=C)
    cap = max(P, math.ceil(counts.max() / P) * P)
    order = np.argsort(cond_ids, kind="stable")
    starts = np.concatenate([[0], np.cumsum(counts)])

    relu_b = np.maximum(b, 0.0)  # [C, D]
    S = relu_b.sum(axis=0)  # [D]

    in_maps = []
    rows_per_core = []
    for c in range(C):
        rows_c = order[starts[c] : starts[c + 1]]
        rows_per_core.append(rows_c)
        xT_c = np.zeros((D, cap), dtype=np.float32)
        if len(rows_c):
            xT_c[:, : len(rows_c)] = x[rows_c].T
        corr_c = S - relu_b[c]  # >= 0 (sum of relus over the other conds)
        in_maps.append(
            {
                "xt": xT_c,
                "w": np.ascontiguousarray(W[c]),
                # device computes max(z + (b + corr), corr)
                "b2": np.ascontiguousarray((b[c] + corr_c).reshape(FT, P).T),
                "corr2": np.ascontiguousarray(corr_c.reshape(FT, P).T),
            }
        )
    return in_maps, rows_per_core, cap


def run(x, cond_ids, W, b, trace: bool = False):
    """Run the kernel; returns (out, BassKernelResults)."""
    try:
        from concourse.bass_utils import run_bass_kernel_spmd
    except ImportError:
        import sys

        sys.path.append("/opt/trn_rl_repo")
        from concourse.bass_utils import run_bass_kernel_spmd

    in_maps, rows_per_core, cap = _route(x, cond_ids, W, b)
    nc = _get_program(cap, MODE)
    res = run_bass_kernel_spmd(
        nc, in_maps, core_ids=list(range(NCORES)), trace=trace
    )

    out = np.empty((len(np.asarray(cond_ids)), D), dtype=np.float32)
    for c in range(C):
        rows_c = rows_per_core[c]
        if len(rows_c):
            out[rows_c] = res.results[c]["yt"][:, : len(rows_c)].T
    return out, res


def kernel(x, cond_ids, W, b):
    out, _ = run(x, cond_ids, W, b, trace=False)
    return out



# revision 45
# speedup vs baseline: 1.0931x; 1.0185x over previous
"""Trainium2 Bass kernel for nn_ConditionalLayer (MoE-style conditional FC).

Reference semantics (N=16384 rows, D=512 features, C=8 conditions):
    out[n] = sum_c relu( (x[n] * [cond_ids[n]==c]) @ W_c + b_c )
           = relu(x[n] @ W_{c*} + b_{c*}) + sum_{c != c*} relu(b_c)
where c* = cond_ids[n].  Masked-out rows still contribute relu(b_c), a
per-condition constant vector corr_{c*} = S - relu(b_{c*}), S = sum_c relu(b_c).

Strategy (expert-parallel, 8 cores == 8 conditions):
  - Host: group rows by condition (argsort), pad each group to a common CAP,
    and ship core c the transposed row-block xT_c = [D, CAP] plus W_c and the
    two per-feature constant vectors (b_c, corr_c).
  - Device (per core): yT = relu(W_c.T-contract xT + b_c) + corr_c as a dense
    [CAP,512]x[512,512] matmul tiled on the 128x128 PE, PSUM fp32 accumulation,
    relu+bias on ScalarE, +corr on VectorE, all overlapped with DMA.
  - Host: scatter rows back to their original positions.

Each row is matmul'd against only its own condition's weights: 1/8th the FLOPs
of the reference's masked-batch formulation.
"""

import math

import numpy as np

N, D, C = 16384, 512, 8
NCORES = 8
P = 128
KT = D // P  # 4 k-tiles
FT = D // P  # 4 output feature tiles
RTILE = 512  # moving free dim per matmul (max for fp32)

# MODE:
#   "f32"         — plain fp32 matmul (4 cycles/row, safest numerics)
#   "f32r_direct" — fp32r matmul, operands DMA'd as raw fp32 bits (1 cycle/row)
#   "f32r_round"  — fp32r matmul, operands pre-rounded via DVE copy
MODE = "f32r_direct"

_PROGRAM_CACHE: dict = {}


def _build_program(cap: int, mode: str):
    import concourse.mybir as mybir
    import concourse.tile as tile
    from concourse import bacc

    f32 = mybir.dt.float32
    mmdt = f32 if mode == "f32" else mybir.dt.float32r

    # Bacc (not Bass): its compile() runs move_matmul_waits_to_ldweights +
    # generate_event_semaphores, required to satisfy the 1-wait-per-instruction
    # HW constraint on fused-weight-load matmuls (fp32/fp32r).
    nc = bacc.Bacc("TRN2", target_bir_lowering=False, debug=False)

    xt = nc.dram_tensor("xt", [D, cap], f32, kind="ExternalInput")
    w = nc.dram_tensor("w", [D, D], f32, kind="ExternalInput")
    b2 = nc.dram_tensor("b2", [P, FT], f32, kind="ExternalInput")
    corr2 = nc.dram_tensor("corr2", [P, FT], f32, kind="ExternalInput")
    yt = nc.dram_tensor("yt", [D, cap], f32, kind="ExternalOutput")

    xt_r = xt[:].rearrange("(kt p) r -> p kt r", p=P)  # [128, KT, cap]
    w_r = w[:].rearrange("(kt p) f -> p kt f", p=P)  # [128, KT, D]
    yt_r = yt[:].rearrange("(ft p) r -> p ft r", p=P)  # [128, FT, cap]

    # r-tile schedule: a small first tile so the PE starts as soon as a
    # small x slice lands, full 512s in the middle, and a 384 last tile so
    # the final store transfer isn't a fully exposed 1.1 MB at the kernel
    # tail.  (Extra tiles below free-dim 256 measured net-negative: more
    # LDWEIGHTS plus fp32r's 4-cycles/row penalty.)  cap is a multiple
    # of 128.
    head, tail = [256], [384]
    if cap >= sum(head) + sum(tail) + RTILE:
        mid = cap - sum(head) - sum(tail)
        sizes = head + [RTILE] * (mid // RTILE)
        if mid % RTILE:
            sizes.append(mid % RTILE)
        sizes += tail
    else:
        sizes = []
        rem = cap
        while rem:
            s = min(RTILE, rem)
            sizes.append(s)
            rem -= s
    r_tiles = []
    off = 0
    for s in sizes:
        r_tiles.append((off, s))
        off += s
    assert off == cap

    direct = mode != "f32r_round"
    load_dt = mmdt if direct else f32

    with tile.TileContext(nc) as tc:
        with (
            tc.tile_pool(name="wpool", bufs=1) as wpool,
            tc.tile_pool(name="cpool", bufs=1) as cpool,
            tc.tile_pool(name="xpool", bufs=4) as xpool,
            tc.tile_pool(name="opool", bufs=3) as opool,
            tc.tile_pool(name="pspool", bufs=8, space="PSUM") as pspool,
        ):
            # Per-queue transfers are FIFO; cross-queue they round-robin.
            # w goes FIRST on the Sync queue ahead of the x-load FIFO
            # (every matmul group needs all of it before its k-accumulation
            # completes); consts + stores go on GpSimd's queue so
            # load-prefetch and store-drain don't serialize on one trigger
            # queue (each dma trigger costs ~600-900ns of issuing-engine
            # time).
            w_sb = wpool.tile([P, KT, D], load_dt)
            src = w_r if load_dt is f32 else w_r.bitcast(load_dt)
            nc.sync.dma_start(w_sb[:], src)
            if not direct:
                w_rnd = wpool.tile([P, KT, D], mmdt)
                nc.vector.tensor_copy(w_rnd[:], w_sb[:])
                w_mm = w_rnd
            else:
                w_mm = w_sb
            b_sb = cpool.tile([P, FT], f32)
            nc.gpsimd.dma_start(b_sb[:], b2[:])
            c_sb = cpool.tile([P, FT], f32)
            nc.gpsimd.dma_start(c_sb[:], corr2[:])

            for roff, rsz in r_tiles:
                x_sb = xpool.tile([P, KT, RTILE], load_dt, tag="x")
                xsrc = xt_r[:, :, roff : roff + rsz]
                if load_dt is not f32:
                    xsrc = xsrc.bitcast(load_dt)
                nc.sync.dma_start(x_sb[:, :, :rsz], xsrc)
                if not direct:
                    x_rnd = xpool.tile([P, KT, RTILE], mmdt, tag="xr")
                    nc.vector.tensor_copy(x_rnd[:, :, :rsz], x_sb[:, :, :rsz])
                    x_mm = x_rnd
                else:
                    x_mm = x_sb
                o_sb = opool.tile([P, FT, RTILE], f32, tag="o")
                for ft in range(FT):
                    ps = pspool.tile([P, RTILE], f32, tag="ps")
                    for kt in range(KT):
                        nc.tensor.matmul(
                            ps[:, :rsz],
                            lhsT=w_mm[:, kt, ft * P : (ft + 1) * P],
                            rhs=x_mm[:, kt, :rsz],
                            start=(kt == 0),
                            stop=(kt == KT - 1),
                        )
                    # relu(z + b) + corr == max(z + (b + corr), corr) since
                    # corr >= 0 — one DVE op instead of an ACT relu plus a
                    # DVE add (b_sb holds b + corr, c_sb holds corr)
                    nc.vector.tensor_scalar(
                        o_sb[:, ft, :rsz],
                        ps[:, :rsz],
                        b_sb[:, ft : ft + 1],
                        c_sb[:, ft : ft + 1],
                        mybir.AluOpType.add,
                        mybir.AluOpType.max,
                    )
                # one batched store per r-tile on GpSimd's queue
                nc.gpsimd.dma_start(
                    yt_r[:, :, roff : roff + rsz], o_sb[:, :, :rsz]
                )

    nc.compile()
    return nc


def _get_program(cap: int, mode: str):
    key = (cap, mode)
    if key not in _PROGRAM_CACHE:
        _PROGRAM_CACHE[key] = _build_program(cap, mode)
    return _PROGRAM_CACHE[key]


def _route(x, cond_ids, W, b):
    """Host-side routing: group rows by condition, build per-core inputs."""
    x = np.ascontiguousarray(np.asarray(x, dtype=np.float32))
    cond_ids = np.asarray(cond_ids, dtype=np.int32)
    W = np.asarray(W, dtype=np.float32)
    b = np.asarray(b, dtype=np.float32)

    counts = np.bincount(cond_ids, minlength=C)
    cap = max(P, math.ceil(counts.max() / P) * P)
    order = np.argsort(cond_ids, kind="stable")
    starts = np.concatenate([[0], np.cumsum(counts)])

    relu_b = np.maximum(b, 0.0)  # [C, D]
    S = relu_b.sum(axis=0)  # [D]

    in_maps = []
    rows_per_core = []
    for c in range(C):
        rows_c = order[starts[c] : starts[c + 1]]
        rows_per_core.append(rows_c)
        xT_c = np.zeros((D, cap), dtype=np.float32)
        if len(rows_c):
            xT_c[:, : len(rows_c)] = x[rows_c].T
        corr_c = S - relu_b[c]  # >= 0 (sum of relus over the other conds)
        in_maps.append(
            {
                "xt": xT_c,
                "w": np.ascontiguousarray(W[c]),
                # device computes max(z + (b + corr), corr)
                "b2": np.ascontiguousarray((b[c] + corr_c).reshape(FT, P).T),
                "corr2": np.ascontiguousarray(corr_c.reshape(FT, P).T),
            }
        )
    return in_maps, rows_per_core, cap


def run(x, cond_ids, W, b, trace: bool = False):
    """Run the kernel; returns (out, BassKernelResults)."""
    try:
        from concourse.bass_utils import run_bass_kernel_spmd
    except ImportError:
        import sys

        sys.path.append("/opt/trn_rl_repo")
        from concourse.bass_utils import run_bass_kernel_spmd

    in_maps, rows_per_core, cap = _route(x, cond_ids, W, b)
    nc = _get_program(cap, MODE)
    res = run_bass_kernel_spmd(
        nc, in_maps, core_ids=list(range(NCORES)), trace=trace
    )

    out = np.empty((len(np.asarray(cond_ids)), D), dtype=np.float32)
    for c in range(C):
        rows_c = rows_per_core[c]
        if len(rows_c):
            out[rows_c] = res.results[c]["yt"][:, : len(rows_c)].T
    return out, res


def kernel(x, cond_ids, W, b):
    out, _ = run(x, cond_ids, W, b, trace=False)
    return out

